# revision 1
# baseline (speedup 1.0000x reference)
"""ConvCapsuleLayer TRN2 kernel v2.

Sharding: 8 cores = B(2) x D-quarters(4); zero cross-core communication.
Per d-plane: conv as fp32r matmuls with kh/kw shifts baked into SBUF tile rows
(6 matmuls per 384-col chunk: K=128 + K=16 leftover, accumulated in PSUM) ->
votes [oc=(nc,na)=128, 2304]. Routing runs fully chunk-local on 384-col chunks
(3x128, transpose-aligned): reductions over na and broadcasts over nc/ic are
0/1-matrix PE matmuls in fp32r; elementwise on DVE; transcendentals + PSUM
evictions on ACT. Small tensors use the zero-padded "L32" layout
[32*ic + nc] so all ops stay partition-aligned. fp32r (~12-bit mantissa)
keeps end-to-end relative error ~2.5e-3.
"""
import sys
import numpy as np

sys.path.insert(0, "/opt/trn_rl_repo")

import concourse.bass as bass
import concourse.mybir as mybir
from concourse import bacc, tile
from contextlib import ExitStack

F32 = mybir.dt.float32
F32R = mybir.dt.float32r
F16 = mybir.dt.float16

B, D, H, W, IC, A = 2, 24, 48, 48, 4, 16
NC, NA = 8, 16
OC = 128
DPC = 6
DSLAB = DPC + 2
HP, WPAD = 50, 52
PLANE_POS = H * W      # 2304
CN = 384               # chunk cols (8 h-rows)
NCH = PLANE_POS // CN  # 6 chunks per plane
CROWS = CN // W        # 8


def build_program(gpsimd_adds=True, post_t_factor=True):
    NT = 1
    nc = bacc.Bacc("TRN2", target_bir_lowering=False, debug=False, num_devices=8)
    xp_e = nc.dram_tensor("xp", [IC, DSLAB, NT, OC, HP * HP], F32R, kind="ExternalInput").ap()
    wA_e = nc.dram_tensor("wA", [3, OC, OC], F32R, kind="ExternalInput").ap()
    wC_e = nc.dram_tensor("wC", [3, OC, OC], F32R, kind="ExternalInput").ap()
    bias_e = nc.dram_tensor("bias", [OC, 1], F32, kind="ExternalInput").ap()
    ena8_e = nc.dram_tensor("ena8", [OC, NC], F32R, kind="ExternalInput").ap()
    edl_e = nc.dram_tensor("edl", [IC, OC, OC], F32R, kind="ExternalInput").ap()
    ebc8_e = nc.dram_tensor("ebc8", [NC, OC], F32R, kind="ExternalInput").ap()
    es32_e = nc.dram_tensor("es32", [OC, IC], F32R, kind="ExternalInput").ap()
    ebc32_e = nc.dram_tensor("ebc32", [IC, OC], F32R, kind="ExternalInput").ap()
    erbc_e = nc.dram_tensor("erbc", [IC, OC, OC], F32R, kind="ExternalInput").ap()
    out_e = nc.dram_tensor("out", [DPC * H * W, OC], F32, kind="ExternalOutput").ap()

    with ExitStack() as ctx:
        tc = ctx.enter_context(tile.TileContext(nc))
        cpool = ctx.enter_context(tc.tile_pool(name="const", bufs=1))
        planep = ctx.enter_context(tc.tile_pool(name="planes", bufs=4))
        votesp = ctx.enter_context(tc.tile_pool(name="votes", bufs=8))
        scr = ctx.enter_context(tc.tile_pool(name="scr", bufs=3))
        scr2 = ctx.enter_context(tc.tile_pool(name="scr2", bufs=3))
        ps_conv = ctx.enter_context(tc.tile_pool(name="ps_conv", bufs=1, space="PSUM"))
        ps_bc = ctx.enter_context(tc.tile_pool(name="ps_bc", bufs=3, space="PSUM"))
        ps_small = ctx.enter_context(tc.tile_pool(name="ps_small", bufs=2, space="PSUM"))
        ps_delta = ctx.enter_context(tc.tile_pool(name="ps_delta", bufs=1, space="PSUM"))
        ps_tr = ctx.enter_context(tc.tile_pool(name="ps_tr", bufs=1, space="PSUM"))

        # --- resident constants ---
        wA_s = cpool.tile([OC, 3, OC], F32R, tag="wA")
        nc.sync.dma_start(out=wA_s[:], in_=wA_e.rearrange("k p m -> p k m"))
        wC_s = cpool.tile([OC, 3, OC], F32R, tag="wC")
        nc.sync.dma_start(out=wC_s[:], in_=wC_e.rearrange("k p m -> p k m"))
        bias_s = cpool.tile([OC, 1], F32, tag="bias")
        nc.sync.dma_start(out=bias_s[:], in_=bias_e[:])
        ena8_s = cpool.tile([OC, NC], F32R, tag="ena8")
        nc.sync.dma_start(out=ena8_s[:], in_=ena8_e[:])
        edl_s = cpool.tile([OC, IC, OC], F32R, tag="edl")
        nc.sync.dma_start(out=edl_s[:], in_=edl_e.rearrange("i k m -> k i m"))
        ebc8_s = cpool.tile([NC, OC], F32R, tag="ebc8")
        nc.sync.dma_start(out=ebc8_s[:], in_=ebc8_e[:])
        es32_s = cpool.tile([OC, IC], F32R, tag="es32")
        nc.sync.dma_start(out=es32_s[:], in_=es32_e[:])
        ebc32_s = cpool.tile([IC, OC], F32R, tag="ebc32")
        nc.sync.dma_start(out=ebc32_s[:], in_=ebc32_e[:])
        erbc_s = cpool.tile([OC, IC, OC], F32R, tag="erbc")
        nc.sync.dma_start(out=erbc_s[:], in_=erbc_e.rearrange("i k m -> k i m"))
        from concourse.masks import make_identity
        ident_s = cpool.tile([OC, OC], F32, tag="ident")
        make_identity(nc, ident_s[:])

        adder = nc.gpsimd if gpsimd_adds else nc.vector

        def squash_factor(pre):
            """pre [128, CN] F32 -> fac [8, CN] F32R = sqrt(n2)/(1+n2)."""
            sq = scr.tile([OC, CN], F32R, tag="sq")
            nc.scalar.square(out=sq[:], in_=pre[:])
            n2 = ps_small.tile([NC, CN], F32, tag="small")
            nc.tensor.matmul(out=n2[:], lhsT=ena8_s[:], rhs=sq[:],
                             start=True, stop=True)
            t1 = scr2.tile([NC, CN], F32, tag="t1")
            nc.scalar.activation(out=t1[:], in_=n2[:],
                                 func=mybir.ActivationFunctionType.Identity,
                                 bias=1.0, scale=1.0)
            rec = scr2.tile([NC, CN], F32, tag="rec")
            nc.vector.reciprocal_approx_fast(out=rec[:], in_=t1[:])
            nrm = scr2.tile([NC, CN], F32, tag="nrm")
            nc.scalar.sqrt(out=nrm[:], in_=n2[:])
            fac = scr2.tile([NC, CN], F32R, tag="fac")
            nc.vector.tensor_mul(out=fac[:], in0=nrm[:], in1=rec[:])
            return fac

        def apply_factor_bc(pre, fac):
            """pre *= bcast(fac) in place."""
            fbc = ps_bc.tile([OC, CN], F32, tag="bc")
            nc.tensor.matmul(out=fbc[:], lhsT=ebc8_s[:], rhs=fac[:],
                             start=True, stop=True)
            nc.vector.tensor_mul(out=pre[:], in0=pre[:], in1=fbc[:])

        def softmax_r(logits):
            """logits [128, CN] L32 F32 -> r [128, CN] F32R."""
            e = scr.tile([OC, CN], F32R, tag="e")
            nc.scalar.activation(out=e[:], in_=logits[:],
                                 func=mybir.ActivationFunctionType.Exp)
            s = ps_small.tile([IC, CN], F32, tag="small")
            nc.tensor.matmul(out=s[:], lhsT=es32_s[:], rhs=e[:],
                             start=True, stop=True)
            rsf = scr2.tile([IC, CN], F32, tag="rsf")
            nc.vector.reciprocal_approx_fast(out=rsf[:], in_=s[:])
            rs = scr2.tile([IC, CN], F32R, tag="rs")
            nc.scalar.copy(out=rs[:], in_=rsf[:])
            rsbc = ps_bc.tile([OC, CN], F32, tag="bc")
            nc.tensor.matmul(out=rsbc[:], lhsT=ebc32_s[:], rhs=rs[:],
                             start=True, stop=True)
            r = scr.tile([OC, CN], F32R, tag="r")
            nc.vector.tensor_mul(out=r[:], in0=e[:], in1=rsbc[:])
            return r

        def weighted_pre(vch, r):
            """pre = sum_ic bcast_ic(r)*votes_ic + bias, [128, CN] F32."""
            p0 = scr.tile([OC, CN], F32, tag="prodA")
            p1 = scr.tile([OC, CN], F32, tag="prodB")
            for icp in range(IC):
                rbc = ps_bc.tile([OC, CN], F32, tag="bc")
                nc.tensor.matmul(out=rbc[:], lhsT=erbc_s[:, icp, :], rhs=r[:],
                                 start=True, stop=True)
                if icp < 2:
                    dst = p0 if icp == 0 else p1
                    nc.vector.tensor_mul(out=dst[:], in0=vch[icp], in1=rbc[:])
                else:
                    dst = p0 if icp == 2 else p1
                    tmp = scr.tile([OC, CN], F32, tag="prodC")
                    nc.vector.tensor_mul(out=tmp[:], in0=vch[icp], in1=rbc[:])
                    adder.tensor_add(out=dst[:], in0=dst[:], in1=tmp[:])
            adder.tensor_add(out=p0[:], in0=p0[:], in1=p1[:])
            pre = scr.tile([OC, CN], F32, tag="pre")
            nc.scalar.activation(out=pre[:], in_=p0[:],
                                 func=mybir.ActivationFunctionType.Identity,
                                 bias=bias_s[:], scale=1.0)
            return pre

        def delta_logits(vch, act):
            """dl [128, CN] L32 PSUM = per-ic sum_na votes*act."""
            dl = ps_delta.tile([OC, CN], F32, tag="dl")
            for icp in range(IC):
                p = scr.tile([OC, CN], F32R, tag=f"dprod{icp % 2}")
                nc.vector.tensor_mul(out=p[:], in0=vch[icp], in1=act[:])
                nc.tensor.matmul(out=dl[:], lhsT=edl_s[:, icp, :], rhs=p[:],
                                 start=(icp == 0), stop=(icp == IC - 1))
            return dl

        # ===================== main loop =====================
        for dp in range(DPC):
            vts = []
            for n in range(IC):
                planes = []
                for sp in range(3):
                    tt = []
                    for j in range(NT):
                        t = planep.tile([OC, HP * HP], F32R, tag="plane")
                        nc.sync.dma_start(out=t[:], in_=xp_e[n, dp + sp, j])
                        tt.append(t)
                    planes.append(tt)
                v = votesp.tile([OC, PLANE_POS], F32, tag="votes")
                vts.append(v)
                nmm = 6
                for c in range(NCH):
                    h0 = c * CROWS
                    pc = ps_conv.tile([OC, CN], F32, tag="conv")
                    mi = 0
                    for kd in range(3):
                        tA = planes[kd][0][:]
                        off = h0 * HP + 1
                        rhsA = bass.AP(tA.tensor, tA.offset + off,
                                       [list(tA.ap[0]), [HP, CROWS], [1, 48]])
                        nc.tensor.matmul(out=pc[:], lhsT=wA_s[:, kd, :],
                                         rhs=rhsA, start=(mi == 0),
                                         stop=(mi == nmm - 1))
                        mi += 1
                        tA32 = tA[32:48, :]
                        rhsC = bass.AP(tA32.tensor, tA32.offset + off + 100,
                                       [list(tA32.ap[0]), [HP, CROWS], [1, 48]])
                        nc.tensor.matmul(out=pc[:], lhsT=wC_s[32:48, kd, :],
                                         rhs=rhsC, start=False,
                                         stop=(mi == nmm - 1))
                        mi += 1
                    nc.scalar.copy(out=v[:, c * CN:(c + 1) * CN], in_=pc[:])

            pos_plane = dp * PLANE_POS
            for c in range(NCH):
                c0 = c * CN
                vch = [vts[n][:, c0:c0 + CN] for n in range(IC)]
                # ---- iter 1 (uniform route) ----
                p0 = scr.tile([OC, CN], F32, tag="prodA")
                p1 = scr.tile([OC, CN], F32, tag="prodB")
                adder.tensor_add(out=p0[:], in0=vch[0], in1=vch[1])
                adder.tensor_add(out=p1[:], in0=vch[2], in1=vch[3])
                adder.tensor_add(out=p0[:], in0=p0[:], in1=p1[:])
                pre1 = scr.tile([OC, CN], F32, tag="pre")
                nc.scalar.activation(out=pre1[:], in_=p0[:],
                                     func=mybir.ActivationFunctionType.Identity,
                                     bias=bias_s[:], scale=0.125)
                fac1 = squash_factor(pre1)
                apply_factor_bc(pre1, fac1)
                dl1 = delta_logits(vch, pre1)
                logits = scr.tile([OC, CN], F32, tag="logits")
                nc.scalar.copy(out=logits[:], in_=dl1[:])
                # ---- iter 2 ----
                r2 = softmax_r(logits)
                pre2 = weighted_pre(vch, r2)
                fac2 = squash_factor(pre2)
                apply_factor_bc(pre2, fac2)
                dl2 = delta_logits(vch, pre2)
                nc.vector.tensor_add(out=logits[:], in0=logits[:], in1=dl2[:])
                # ---- iter 3 ----
                r3 = softmax_r(logits)
                pre3 = weighted_pre(vch, r3)
                fac3 = squash_factor(pre3)
                if post_t_factor:
                    f3t = scr2.tile([OC, 3, NC], F32, tag="f3t")
                    for t0 in range(3):
                        trf = ps_tr.tile([OC, OC], F32, tag="tr")
                        facs = fac3[:, 128 * t0:128 * t0 + 128]
                        nc.tensor.transpose(trf[0:128, 0:8], facs.bitcast(F32),
                                            ident_s[0:8, 0:8])
                        nc.scalar.copy(out=f3t[:, t0, :], in_=trf[0:128, 0:8])
                    for t0 in range(3):
                        trp = ps_tr.tile([OC, OC], F32, tag="tr")
                        nc.tensor.transpose(trp[:, :],
                                            pre3[:, 128 * t0:128 * t0 + 128],
                                            ident_s[:])
                        ov = scr2.tile([OC, OC], F32, tag="ov")
                        f3s = f3t[:, t0, :]
                        f3b = bass.AP(f3s.tensor, f3s.offset,
                                      [list(f3s.ap[0]), list(f3s.ap[-1]), [0, NA]])
                        trp3 = trp[:].rearrange("p (b a) -> p b a", a=NA)
                        ov3 = ov[:].rearrange("p (b a) -> p b a", a=NA)
                        nc.vector.tensor_mul(out=ov3, in0=trp3, in1=f3b)
                        pb = pos_plane + c0 + 128 * t0
                        nc.sync.dma_start(out=out_e[pb:pb + 128, :], in_=ov[:])
                else:
                    apply_factor_bc(pre3, fac3)
                    for t0 in range(3):
                        trp = ps_tr.tile([OC, OC], F32, tag="tr")
                        nc.tensor.transpose(trp[:, :],
                                            pre3[:, 128 * t0:128 * t0 + 128],
                                            ident_s[:])
                        ov = scr2.tile([OC, OC], F32, tag="ov")
                        nc.scalar.copy(out=ov[:], in_=trp[:])
                        pb = pos_plane + c0 + 128 * t0
                        nc.sync.dma_start(out=out_e[pb:pb + 128, :], in_=ov[:])

    nc.compile()
    return nc


# ===================== host side =====================

def _trunc10(x):
    xi = np.ascontiguousarray(x, np.float32).view(np.uint32) & np.uint32(0xFFFFE000)
    return xi.view(np.float32)


def prep_inputs(x, conv_w, b):
    x = np.asarray(x, np.float32)
    conv_w = np.asarray(conv_w, np.float32)
    b = np.asarray(b, np.float32)
    NT = 1

    wt = np.zeros((9, 48, OC), np.float32)
    for kd in range(3):
        for kh in range(3):
            for kw in range(3):
                wt[3 * kd + kh, 16 * kw:16 * kw + 16, :] = conv_w[:, :, kd, kh, kw].T
    wA = np.zeros((3, OC, OC), np.float32)
    wC = np.zeros((3, OC, OC), np.float32)
    for kd in range(3):
        wA[kd, 0:48] = wt[3 * kd + 0]
        wA[kd, 48:96] = wt[3 * kd + 1]
        wA[kd, 96:128] = wt[3 * kd + 2, 0:32]
        wC[kd, 32:48] = wt[3 * kd + 2, 32:48]

    bias = b[0, 0, 0].reshape(OC, 1).astype(np.float32)

    ena8 = np.zeros((OC, NC), np.float32)
    ebc8 = np.zeros((NC, OC), np.float32)
    es32 = np.zeros((OC, IC), np.float32)
    ebc32 = np.zeros((IC, OC), np.float32)
    erbc = np.zeros((IC, OC, OC), np.float32)
    edl = np.zeros((IC, OC, OC), np.float32)
    for ncp in range(NC):
        for na in range(NA):
            oc = 16 * ncp + na
            ena8[oc, ncp] = 1.0
            ebc8[ncp, oc] = 1.0
            for ic in range(IC):
                edl[ic, oc, 32 * ic + ncp] = 1.0
    for ic in range(IC):
        for ncp in range(NC):
            l32 = 32 * ic + ncp
            es32[l32, ic] = 1.0
            ebc32[ic, l32] = 1.0
            for na in range(NA):
                erbc[ic, l32, 16 * ncp + na] = 1.0

    xt = np.transpose(x, (0, 4, 5, 1, 2, 3))  # [B, ICg, A, D, H, W]

    def pack_tiles(xpad, xp_core, ic):
        """xpad [A, DSLAB, 52, 52] -> T_A rows into xp_core[ic]."""
        for kh in range(3):
            for kw in range(3):
                if kh == 2 and kw == 2:
                    continue
                r0 = kh * 48 + kw * 16 if kh < 2 else 96 + kw * 16
                blk = xpad[:, :, kh:kh + HP, kw:kw + HP]
                xp_core[ic, :, 0, r0:r0 + 16] = np.transpose(blk, (1, 0, 2, 3))

    in_maps = []
    for core in range(8):
        bc, dq = core // 4, core % 4
        d0 = dq * DPC - 1
        xp = np.zeros((IC, DSLAB, NT, OC, HP, HP), np.float32)
        for ic in range(IC):
            n_g = 4 * bc + ic
            bp, icp = n_g % 2, n_g // 2
            xpad = np.zeros((A, DSLAB, 52, 52), np.float32)
            lo, hi = max(0, d0), min(D, d0 + DSLAB)
            xpad[:, lo - d0:hi - d0, 1:49, 2:50] = xt[bp, icp, :, lo:hi]
            pack_tiles(xpad, xp, ic)
        in_maps.append(dict(xp=np.ascontiguousarray(xp.reshape(IC, DSLAB, NT, OC, HP * HP)),
                            wA=wA, wC=wC, bias=bias, ena8=ena8, edl=edl,
                            ebc8=ebc8, es32=es32, ebc32=ebc32, erbc=erbc))
    return in_maps


def assemble_output(results):
    out = np.zeros((B, D, H, W, NC, NA), np.float32)
    for core in range(8):
        bc, dq = core // 4, core % 4
        r = results[core]["out"].reshape(DPC, H, W, NC, NA)
        out[bc, dq * DPC:(dq + 1) * DPC] = r
    return out


_CACHED = {}


def run(x, conv_w, b, gpsimd_adds=True, post_t_factor=True, trace=False):
    from concourse.bass_utils import run_bass_kernel_spmd
    key = (gpsimd_adds, post_t_factor)
    if key not in _CACHED:
        _CACHED[key] = build_program(gpsimd_adds, post_t_factor)
    nc = _CACHED[key]
    in_maps = prep_inputs(x, conv_w, b)
    res = run_bass_kernel_spmd(nc, in_maps, list(range(8)), trace=trace)
    return assemble_output(res.results), res


# ===================== harness entry point =====================

_NC_PROG = None


def _get_prog():
    global _NC_PROG
    if _NC_PROG is None:
        _NC_PROG = build_program(gpsimd_adds=False, post_t_factor=False)
    return _NC_PROG


def kernel(x, conv_w, b):
    """Full (unsharded) inputs -> full output [2, 24, 48, 48, 8, 16] fp32.

    Shards across 8 NeuronCores internally (batch x depth-quarters),
    runs the Bass kernel via SPMD, gathers per-core outputs.
    """
    from concourse.bass_utils import run_bass_kernel_spmd
    nc = _get_prog()
    in_maps = prep_inputs(x, conv_w, b)
    res = run_bass_kernel_spmd(nc, in_maps, list(range(8)))
    out = assemble_output(res.results)
    return out.astype(np.float32)


def run_traced(x, conv_w, b):
    """Like kernel() but with NTFF tracing; returns (output, BassKernelResults)."""
    try:
        import antenv.axon_hooks as ah
        from trn_agent_boot.trn_boot import _ntff_profile_via_ctypes
        if ah.get_axon_ntff_profile_hook() is None:
            ah.set_axon_ntff_profile_hook(
                _ntff_profile_via_ctypes("/opt/axon/libaxon_pjrt.so"))
    except Exception:
        pass
    from concourse.bass_utils import run_bass_kernel_spmd
    nc = _get_prog()
    in_maps = prep_inputs(x, conv_w, b)
    res = run_bass_kernel_spmd(nc, in_maps, list(range(8)), trace=True)
    return assemble_output(res.results).astype(np.float32), res

